# revision 72
# baseline (speedup 1.0000x reference)
"""Multi-head causal attention (B=2, S=2048, E=1024, H=16, Dh=64) on 8 TRN2
NeuronCores.

Sharding: core c handles batch c//4 and the 4 heads [4*(c%4), 4*(c%4)+4).
Each core computes its heads' QKV projections, causal softmax attention, and
a partial output projection (contraction over its 256 d_inner columns).
The host sums the 4 partial outputs per batch (the "all-reduce") and adds bo.

Device layout notes:
  - Activations enter as X^T (E-major) so every matmul contracts over the
    partition dim.  Q,K are produced transposed (d-major [d, s]); V is
    produced seq-major [s, d]; attention scores are computed transposed
    [k, q] so softmax normalization is a matmul-reduction over partitions.
  - All matmuls run in float32r (full-rate fp32 mode, ~1e-4 component
    rounding).  PSUM accumulation is fp32.
"""

import numpy as np

import concourse.bass as bass
import concourse.tile as tile
from concourse import bacc, mybir
from concourse.bass_utils import run_bass_kernel_spmd

F32 = mybir.dt.float32
F32R = mybir.dt.float32r

B, S, E = 2, 2048, 1024
H, DH = 16, 64
NCORES = 8
HPC = 4          # heads per core
DL = HPC * DH    # 256: d_inner slice per core
NKT = E // 128   # 8  k-tiles over embed dim
NST = S // 128   # 16 seq tiles of 128
NSB = S // 512   # 4  seq blocks of 512
NEG = -1.0e30

ExpF = mybir.ActivationFunctionType.Exp


def build_nc():
    nc = bacc.Bacc("TRN2", target_bir_lowering=False)

    xt_d = nc.dram_tensor("xt", [E, S], F32R, kind="ExternalInput")
    wq_d = nc.dram_tensor("wq", [E, DL], F32R, kind="ExternalInput")
    wk_d = nc.dram_tensor("wk", [E, DL], F32R, kind="ExternalInput")
    wv_d = nc.dram_tensor("wv", [E, DL], F32R, kind="ExternalInput")
    wo_d = nc.dram_tensor("wo", [DL, E], F32R, kind="ExternalInput")
    bqc_d = nc.dram_tensor("bqc", [DL, 1], F32, kind="ExternalInput")
    bkc_d = nc.dram_tensor("bkc", [DL, 1], F32, kind="ExternalInput")
    bv_d = nc.dram_tensor("bv", [1, DL], F32R, kind="ExternalInput")
    ones_d = nc.dram_tensor("ones2d", [65, 512], F32R, kind="ExternalInput")
    vone_d = nc.dram_tensor("v1ones", [128, 2 * HPC], F32R,
                            kind="ExternalInput")
    id_d = nc.dram_tensor("ident", [128, 128], F32R, kind="ExternalInput")
    mask_d = nc.dram_tensor("masks", [128, 4, 1024], F32R,
                            kind="ExternalInput")
    out_d = nc.dram_tensor("out", [E, S], F32, kind="ExternalOutput")

    with tile.TileContext(nc) as tc:
        with tc.tile_pool(name="const", bufs=1) as cp:
            bqc = [cp.tile([128, 1], F32, tag=f"bqc{m}", name=f"bqc{m}")
                   for m in range(2)]
            bkc = [cp.tile([128, 1], F32, tag=f"bkc{m}", name=f"bkc{m}")
                   for m in range(2)]
            bv = cp.tile([1, DL], F32R, tag="bv")
            ones2 = cp.tile([65, 512], F32R, tag="ones2")

            qt = [cp.tile([128, S], F32R, tag=f"qt{m}", name=f"qt{m}")
                  for m in range(2)]
            kt = [cp.tile([128, S], F32R, tag=f"kt{m}", name=f"kt{m}")
                  for m in range(2)]
            ot = [cp.tile([128, S], F32R, tag=f"ot{m}", name=f"ot{m}")
                  for m in range(2)]
            v1 = [cp.tile([128, 2 * HPC * 65], F32R, tag=f"v1{s}",
                          name=f"v1{s}") for s in range(NST // 2)]
            wvt = cp.tile([128, NKT, DL], F32R, tag="wvt")
            wv = [wvt[:, k, :] for k in range(NKT)]
            wo = [cp.tile([128, E], F32R, tag=f"wo{d}", name=f"wo{d}")
                  for d in range(2)]
            ident = cp.tile([128, 128], F32R, tag="ident")
            maskt = cp.tile([128, 4, 1024], F32R, tag="maskt")
            masks = [maskt[:, j, :] for j in range(4)]

            # ============ phase 1 (xt resident) ============
            with (
                tc.tile_pool(name="xtp", bufs=1) as xp,
                tc.tile_pool(name="wstream", bufs=3) as wp,
                tc.tile_pool(name="psqk", bufs=2, space="PSUM") as pqk,
            ):
                xt = [xp.tile([128, S], F32R, tag=f"xt{k}", name=f"xt{k}")
                      for k in range(NKT)]

                # -- Q and K projections, interleaved under the xt stream --
                # qt[m][d, s] = ((X @ Wq + bq).T)[m*128:(m+1)*128, :]
                def qk_proj(m):
                    pq = [pqk.tile([128, 1024], F32, tag="pq", name="pq")
                          for _ in range(2)]
                    pk = [pqk.tile([128, 1024], F32, tag="pk", name="pk")
                          for _ in range(2)]
                    wtq = wp.tile([128, NKT, 128], F32R, tag="wqk",
                                  name="wtq")
                    nc.sync.dma_start(
                        out=wtq[:],
                        in_=wq_d[:, m * 128:(m + 1) * 128].rearrange(
                            "(k p) c -> p k c", p=128))
                    wtk = wp.tile([128, NKT, 128], F32R, tag="wqk",
                                  name="wtk")
                    nc.sync.dma_start(
                        out=wtk[:],
                        in_=wk_d[:, m * 128:(m + 1) * 128].rearrange(
                            "(k p) c -> p k c", p=128))
                    for k in range(NKT):
                        if m == 0:
                            nc.sync.dma_start(
                                out=xt[k][:],
                                in_=xt_d[k * 128:(k + 1) * 128, :])
                            if k == 0:
                                # small consts ride the queue here, off the
                                # first-matmul critical path
                                for mm in range(2):
                                    nc.sync.dma_start(
                                        out=bqc[mm][:],
                                        in_=bqc_d[mm * 128:(mm + 1) * 128, :])
                                    nc.sync.dma_start(
                                        out=bkc[mm][:],
                                        in_=bkc_d[mm * 128:(mm + 1) * 128, :])
                                nc.sync.dma_start(out=bv[:], in_=bv_d[:])
                                nc.sync.dma_start(out=ones2[:],
                                                  in_=ones_d[:])
                        for (wt, ps) in ((wtq, pq), (wtk, pk)):
                            for half in range(2):
                                for loc in range(2):
                                    sb = half * 2 + loc
                                    nc.tensor.matmul(
                                        ps[half][:,
                                                 loc * 512:(loc + 1) * 512],
                                        wt[:, k, :],
                                        xt[k][:, sb * 512:(sb + 1) * 512],
                                        start=(k == 0), stop=(k == NKT - 1),
                                    )
                    for (dst, ps, bc) in ((qt, pq, bqc), (kt, pk, bkc)):
                        nc.scalar.activation(
                            dst[m][:, 0:1024], ps[0][:],
                            mybir.ActivationFunctionType.Identity,
                            bias=bc[m][:],
                        )
                        with nc.allow_low_precision(
                                reason="f32r round of q/k + bias"):
                            nc.vector.tensor_scalar_add(
                                dst[m][:, 1024:2048], ps[1][:], bc[m][:])

                qk_proj(0)

                # remaining constants go on the queue behind xt
                nc.sync.dma_start(
                    out=wvt[:],
                    in_=wv_d.rearrange("(k p) c -> p k c", p=128))
                nc.sync.dma_start(out=ident[:], in_=id_d[:])
                nc.sync.dma_start(out=maskt[:], in_=mask_d[:])
                for s in range(NST // 2):
                    nc.sync.dma_start(
                        out=v1[s].rearrange("p (h c) -> p h c",
                                            c=65)[:, :, 64:65],
                        in_=vone_d.rearrange("p (h c) -> p h c", c=1)[:],
                    )
                for d in range(2):
                    nc.sync.dma_start(out=wo[d][:],
                                      in_=wo_d[d * 128:(d + 1) * 128, :])

                # -- V projection (activations stationary) --
                # v1[st][s, 65h:65h+64] = (X @ Wv + bv)[st*128.., 64h:64h+64]
                for sp2 in range(NST // 2):
                    pv = pqk.tile([128, 512], F32,
                                  tag=("pq" if sp2 % 2 else "pk"), name="pv")
                    for i in range(2):
                        st = 2 * sp2 + i
                        nc.tensor.matmul(pv[:, i * DL:(i + 1) * DL],
                                         ones2[0:1, 0:128], bv[:],
                                         start=True, stop=False)
                        for k in range(NKT):
                            nc.tensor.matmul(
                                pv[:, i * DL:(i + 1) * DL],
                                xt[k][:, st * 128:(st + 1) * 128],
                                wv[k],
                                start=False, stop=(k == NKT - 1),
                            )
                    if sp2 % 2:
                        nc.scalar.copy(
                            out=v1[sp2].rearrange("p (h c) -> p h c",
                                                  c=65)[:, :, 0:64],
                            in_=pv[:].rearrange("p (h c) -> p h c", c=64),
                        )
                    else:
                        nc.vector.tensor_copy(
                            v1[sp2].rearrange("p (h c) -> p h c",
                                              c=65)[:, :, 0:64],
                            pv[:].rearrange("p (h c) -> p h c", c=64),
                        )


                qk_proj(1)

            # ============ phase 2 (xt region reused) ============
            with (
                tc.tile_pool(name="ptp", bufs=16) as ptp,
                tc.tile_pool(name="small", bufs=4) as sp,
                tc.tile_pool(name="oev", bufs=6) as op,
                tc.tile_pool(name="psatt", bufs=1, space="PSUM") as pat,
            ):
                # -- attention + interleaved output projection --
                # out[e, sb*512:+512] partial = Wo_slice.T @ concat_d(ot);
                # each 512-wide s block is emitted as soon as its q block
                # finishes (both head pairs), filling PE gaps.
                def oproj_sb(sb, et):
                    p3 = pat.tile([128, 512], F32, tag="po", bufs=4,
                                  name="p3")
                    for d in range(2):
                        nc.tensor.matmul(
                            p3[:],
                            wo[d][:, et * 128:(et + 1) * 128],
                            ot[d][:, sb * 512:(sb + 1) * 512],
                            start=(d == 0), stop=(d == 1),
                        )
                    oe = op.tile([128, 512], F32, tag="oe", name="oe")
                    if et % 2 == 0:
                        nc.scalar.copy(out=oe[:], in_=p3[:])
                    else:
                        nc.vector.tensor_copy(oe[:], p3[:])
                    nc.sync.dma_start(
                        out=out_d[et * 128:(et + 1) * 128,
                                  sb * 512:(sb + 1) * 512],
                        in_=oe[:],
                    )

                for qb in range(NSB):
                    q0 = qb * 512
                    nkb = 4 * qb + 4
                    for hp in range(2):
                        po_a = pat.tile([65, 512], F32, tag="po", bufs=4,
                                        name="po_a")
                        po_b = pat.tile([65, 512], F32, tag="po", bufs=4,
                                        name="po_b")
                        pending = []
                        for kb in range(nkb):
                            stile = pat.tile([128, 1024], F32, tag="st",
                                             bufs=2, name="stile")
                            j = kb - 4 * qb
                            for h in range(2):
                                nc.tensor.matmul(
                                    stile[:, h * 512:(h + 1) * 512],
                                    kt[hp][h * 64:(h + 1) * 64,
                                           kb * 128:(kb + 1) * 128],
                                    qt[hp][h * 64:(h + 1) * 64, q0:q0 + 512],
                                    start=True, stop=True,
                                )
                            # PV trails ST by up to 2 blocks so the PE
                            # queue never blocks on exp
                            if len(pending) >= 6:
                                pkb, ppt, pfirst, plast = pending.pop(0)
                                for h, po in ((0, po_a), (1, po_b)):
                                    lh = 2 * hp + h
                                    nc.tensor.matmul(
                                        po[:],
                                        v1[pkb // 2][:,
                                            (pkb % 2) * 260 + lh * 65:
                                            (pkb % 2) * 260 + (lh + 1) * 65],
                                        ppt[:, h * 512:(h + 1) * 512],
                                        start=pfirst, stop=plast,
                                    )
                            pt = ptp.tile([128, 1024], F32R, tag="pt",
                                          name="pt")
                            nc.scalar.activation(pt[:], stile[:], ExpF,
                                                 scale=0.125)
                            if j >= 0:
                                with nc.allow_low_precision(
                                        reason="0/1 mask multiply"):
                                    nc.vector.tensor_mul(pt[:], pt[:],
                                                         masks[j])
                            pending.append((kb, pt, kb == 0,
                                            kb == nkb - 1))
                        for pkb, ppt, pfirst, plast in pending:
                            for h, po in ((0, po_a), (1, po_b)):
                                lh = 2 * hp + h
                                nc.tensor.matmul(
                                    po[:],
                                    v1[pkb // 2][:,
                                        (pkb % 2) * 260 + lh * 65:
                                        (pkb % 2) * 260 + (lh + 1) * 65],
                                    ppt[:, h * 512:(h + 1) * 512],
                                    start=pfirst, stop=plast,
                                )
                        # normalize: ot[hp][h*64.., q0:+512] = po[0:64]/po[64]
                        for h, po in ((0, po_a), (1, po_b)):
                            rb = sp.tile([65, 512], F32R, tag="rb", name="rb")
                            with nc.allow_low_precision(
                                    reason="f32r rounding of softmax denoms"):
                                nc.vector.reciprocal(rb[64:65, :],
                                                     po[64:65, :])
                            pb = pat.tile([64, 512], F32, tag="po", bufs=4,
                                          name="pb")
                            nc.tensor.matmul(pb[:], ones2[64:65, 0:64],
                                             rb[64:65, :],
                                             start=True, stop=True)
                            rbb = sp.tile([64, 512], F32, tag="rbb",
                                          name="rbb")
                            nc.vector.tensor_copy(rbb[:], pb[:])
                            if h == 0:
                                nc.vector.tensor_mul(
                                    ot[hp][0:64, q0:q0 + 512],
                                    po[0:64, :], rbb[:])
                            else:
                                tmp = sp.tile([64, 512], F32R, tag="tmp",
                                              name="tmp")
                                nc.vector.tensor_mul(tmp[:], po[0:64, :],
                                                     rbb[:])
                                nc.sync.dma_start(
                                    out=ot[hp][64:128, q0:q0 + 512],
                                    in_=tmp[:])
                        # emit the previous q block's output projection
                        # here (deps long satisfied; fills PE gaps)
                        if hp == 0 and qb > 0:
                            for et in range(NKT):
                                oproj_sb(qb - 1, et)
                for et in range(NKT):
                    oproj_sb(NSB - 1, et)

    nc.compile()
    return nc


def _make_masks():
    kk = np.arange(128)[:, None]
    qq = np.arange(512)[None, :]
    ms = []
    for j in range(4):
        m = np.where(qq >= kk + 128 * j, 1.0, 0.0).astype(np.float32)
        ms.append(np.concatenate([m, m], axis=1))
    return np.ascontiguousarray(np.stack(ms).transpose(1, 0, 2))  # [128,4,1024]


_NC = None


def _get_nc():
    global _NC
    if _NC is None:
        _NC = build_nc()
    return _NC


def make_in_maps(inputs, Wq, bq, Wk, bk, Wv, bv, Wo):
    masks = _make_masks()
    ones2 = np.ones((65, 512), np.float32)
    vones = np.ones((128, 2 * HPC), np.float32)
    ident = np.eye(128, dtype=np.float32)
    in_maps = []
    for c in range(NCORES):
        b, g = c // HPC, c % HPC
        sl = slice(g * DL, (g + 1) * DL)
        in_maps.append({
            "xt": np.ascontiguousarray(np.asarray(inputs[b]).T),
            "wq": np.ascontiguousarray(Wq[:, sl]),
            "wk": np.ascontiguousarray(Wk[:, sl]),
            "wv": np.ascontiguousarray(Wv[:, sl]),
            "wo": np.ascontiguousarray(Wo[sl, :]),
            "bqc": bq[sl].reshape(DL, 1),
            "bkc": bk[sl].reshape(DL, 1),
            "bv": bv[sl].reshape(1, DL),
            "ones2d": ones2,
            "v1ones": vones,
            "ident": ident,
            "masks": masks,
        })
    return in_maps


def assemble(results):
    outs = []
    for b in range(B):
        acc = results[b * HPC]["out"].astype(np.float32).copy()
        for g in range(1, HPC):
            acc += results[b * HPC + g]["out"]
        outs.append(acc.T)
    return np.stack(outs)


def kernel(inputs, Wq, bq, Wk, bk, Wv, bv, Wo, bo):
    inputs = np.asarray(inputs, np.float32)
    args = [np.asarray(a, np.float32) for a in (Wq, bq, Wk, bk, Wv, bv, Wo)]
    in_maps = make_in_maps(inputs, *args)
    nc = _get_nc()
    res = run_bass_kernel_spmd(nc, in_maps, list(range(NCORES)))
    out = assemble(res.results)
    return (out + np.asarray(bo, np.float32)).astype(np.float32)
